# revision 49
# baseline (speedup 1.0000x reference)
"""Trainium2 Bass kernel for the non-local attention block (nn_Attention_79809082295188).

Reference computation (per batch b of 4, C=512 channels, N=4096 positions):
    theta = W_theta @ x          [64, N]
    phi   = W_phi @ x            [64, N]
    g     = W_g @ x              [256, N]
    scores[n, m] = theta[:, n] . phi[:, m]
    beta = softmax(scores, axis=m)
    o_mid[c, n] = sum_m g[c, m] beta[n, m]
    out = gamma * (W_o @ o_mid) + x

Sharding: 8 shards = batch(4) x query-half(2). Each core receives its batch's
full x with its own query half permuted to the FIRST 2048 columns (key order is
irrelevant to softmax attention), computes attention for those 2048 queries
against all 4096 keys, and writes a [512, 2048] output chunk.

On-core dataflow (fp8 DoubleRow everywhere the PE allows it):
  - x ships as fp8e4m3 (2MB) for every projection; a bf16 copy of the core's
    own query half (xq) ships only for the residual add. Output is bf16.
  - theta/phi/g projections run fp8 DoubleRow: channel-block pairs ride the
    interleave dim, halving the moving-element cost vs bf16. Weights are
    pre-scaled by 64 on the host so fp8e4m3 stays in its normal range; the
    theta/phi scale is undone by a 1/4096 fold into the phi2 PSUM->SBUF copy
    (NOT via the exp activation's scale param -- a non-unit activation scale
    measured ~20% slower per exp on HW), the g/o scales cancel in the final
    residual multiplier gamma/2048.
  - wqk/wph/wg ship as ONE packed dram param (wk3): each dma_start costs
    ~0.7us of serialized Sync-engine descriptor generation, and the input
    DMA chain gates the pipeline head. Warmup matmuls (16 x 256-col) bridge
    the PE from boot to first-dependency-arrival; any PE idle gap in the
    head drops the clock (observed 630-690ns vs 380ns for the same matmul).
  - scores are computed TRANSPOSED ([keys m on partitions, queries n free])
    in bf16 with two 64-deep key-chunk matmuls packed onto the 128-row PE via
    tile_position row groups (measured: the pair co-streams, ~213ns/512q).
  - softmax denominator comes for free: a constant column appended to g^T
    makes column 256 of the PV matmul output equal 2*sum_m exp(scores_T[m,n]),
    which also lands o_mid at 32x (safe mid-range for its fp8 recast).
  - max-subtraction is skipped: scores are in [-11, 11], exp() is safe in fp32;
    exp is emitted as exp(s)*2^-9 so it fits fp8e4 range.
  - normalization is a per-partition scalar multiply, then a PE transpose of
    the [queries, 256] result back to [channels, queries] for the fp8-DR
    output projection. (A transpose-free oproj via is_transpose with a
    non-identity rhs does NOT work: the transpose path moves data without
    MACs, so it cannot contract -- produces garbage.)
"""

import sys

sys.path.insert(0, "/opt/trn_rl_repo")

from contextlib import ExitStack

import numpy as np
import ml_dtypes

import concourse.bass as bass
import concourse.bacc as bacc
import concourse.tile as tile
from concourse import mybir
from concourse.bass_utils import run_bass_kernel_spmd
from concourse.masks import make_identity

F32 = mybir.dt.float32
BF16 = mybir.dt.bfloat16
F8 = mybir.dt.float8e4
DR = mybir.MatmulPerfMode.DoubleRow

# exp() is emitted as exp(s)*2^-EXP_SHIFT so it fits fp8e4 range (max ~240
# vs exp(score_max~11) ~ 60000); the scale cancels in the softmax ratio.
EXP_SHIFT = 9
EXP_BIAS = -float(EXP_SHIFT) * 0.6931471805599453
GT_STRIDE = 272  # g^T row stride in fp8 bytes: 257 columns padded to %16==0

SW = 64.0        # host scale on W_theta/W_phi (fp8 normal range)
SG = 64.0        # host scale on W_g
SO = 64.0        # host scale on W_o
ONES = 2.0       # gt denominator column value; omid lands at SG/ONES = 32x

C = 512          # channels
N = 4096         # sequence positions (keys per core)
P = 128          # partitions
CB = C // P      # 4 channel blocks
KD = 64          # theta/phi dim (C/8)
VD = 256         # g dim (C/2)
NQ = 2048        # queries per core
QB = 512         # query block
NQB = NQ // QB   # 4 query blocks
MT = N // P      # 32 key tiles
NCOL = 4         # x column tiles (for DMA/compute overlap)
COLW = N // NCOL # 1024
N_WARMUP = 18    # PE warmup matmuls to ride out the input DMA + HAM cold clock


def build_nc(gamma: float) -> bass.Bass:
    resid_scale = float(gamma) / (SO * SG / ONES)
    nc = bacc.Bacc(
        "TRN2",
        target_bir_lowering=False,
        debug=False,
        enable_asserts=False,
        num_devices=8,
    )
    x_in = nc.declare_dram_parameter("x", [C, N], F8, isOutput=False)
    xq_in = nc.declare_dram_parameter("xq", [C, NQ], BF16, isOutput=False)
    # wk3 packs [wqk | wph | wg] along columns so ONE dma_start covers all
    # early weights (each dma_start costs ~0.7us of serialized Sync-engine
    # descriptor generation, which was gating the pipeline head).
    #   cols   0:128  wqk: [W_theta^T | W_theta^T]
    #   cols 128:384  wph: [W_phi^T | 0] then [0 | W_phi^T] -- lets the
    #     even/odd key-chunk projections land on partitions 0:64 / 64:128 of
    #     one PSUM tile via accumulation (walrus rejects col-tiled dst base 64)
    #   cols 384:640  wg
    wk3_in = nc.declare_dram_parameter("wk3", [C, 5 * P], F8, isOutput=False)
    wo_in = nc.declare_dram_parameter("wo", [VD, C], F8, isOutput=False)
    out_ext = nc.declare_dram_parameter("out", [C, NQ], BF16, isOutput=True)

    x_r = x_in.rearrange("(cb p) (j w) -> p cb j w", p=P, w=COLW)
    xq_r = xq_in.rearrange("(cb p) n -> p cb n", p=P)
    out_r = out_ext.rearrange("(cb p) n -> p cb n", p=P)

    with tile.TileContext(nc) as tc, ExitStack() as ctx:
        const = ctx.enter_context(tc.tile_pool(name="const", bufs=1))
        big = ctx.enter_context(tc.tile_pool(name="big", bufs=1))
        eb = ctx.enter_context(tc.tile_pool(name="eb", bufs=2))
        wk = ctx.enter_context(tc.tile_pool(name="wk", bufs=2))
        recp = ctx.enter_context(tc.tile_pool(name="recp", bufs=4))
        outp = ctx.enter_context(tc.tile_pool(name="outp", bufs=4))
        # PSUM budget (8 banks): scores pairs 2x2 + small 2 + oproj 2
        psS = ctx.enter_context(tc.tile_pool(name="psS", bufs=2, space="PSUM"))
        psP = ctx.enter_context(tc.tile_pool(name="psP", bufs=2, space="PSUM"))
        psQ = ctx.enter_context(tc.tile_pool(name="psQ", bufs=2, space="PSUM"))

        # ---- PE warmup: keep TensorE busy during input DMA so HAM unthrottles
        dummy = const.tile([P, QB], BF16, tag="dummy")
        nc.gpsimd.memset(dummy, 0.0)
        # load the exp table-set during the DMA window, not at first real exp
        warm_exp = const.tile([P, 1], F32, tag="warm_exp")
        nc.scalar.activation(
            out=warm_exp,
            in_=dummy[:, 0:1],
            func=mybir.ActivationFunctionType.Exp,
        )
        for i in range(N_WARMUP):
            psw = psS.tile([P, 2 * QB], F32, tag="scores")
            nc.tensor.matmul(
                psw[:, 0 : 2 * P],
                lhsT=dummy[:, 0:P],
                rhs=dummy[:, 0 : 2 * P],
                start=True,
                stop=True,
            )

        # ---- inputs: interleave x column tiles with the weights so the
        # first projection work unblocks as early as possible (wo last) ----
        xf = [
            big.tile([P, CB, COLW], F8, tag=f"xf{j}", name=f"xf{j}")
            for j in range(NCOL)
        ]
        xq = big.tile([P, CB, NQ], BF16, tag="xq")
        wk3_sb = const.tile([P, CB, 5 * P], F8, tag="wk3")
        wo_sb = const.tile([P, 2, C], F8, tag="wo")

        # wk3 (small) first so its transfer doesn't queue behind xf0's 1.5us
        nc.sync.dma_start(out=wk3_sb, in_=wk3_in.rearrange("(cb p) k -> p cb k", p=P))
        nc.sync.dma_start(out=xf[0], in_=x_r[:, :, 0, :])
        nc.sync.dma_start(out=xf[1], in_=x_r[:, :, 1, :])
        nc.sync.dma_start(out=xf[2], in_=x_r[:, :, 2, :])
        nc.sync.dma_start(out=xf[3], in_=x_r[:, :, 3, :])
        nc.sync.dma_start(out=wo_sb, in_=wo_in.rearrange("(cb p) k -> p cb k", p=P))
        nc.sync.dma_start(out=xq, in_=xq_r)
        ident = const.tile([P, P], BF16, tag="ident")
        make_identity(nc, ident)
        exp_bias = const.tile([P, 1], F32, tag="exp_bias")
        nc.vector.memset(exp_bias, EXP_BIAS)

        def xcols(lo, hi):
            """AP for x columns [lo, hi) -- must lie within one column tile."""
            j = lo // COLW
            assert hi <= (j + 1) * COLW
            return xf[j][:, :, lo - j * COLW : hi - j * COLW]

        # theta duplicated on both partition halves (for row-packed QK^T)
        theta2 = big.tile([P, NQ], BF16, tag="theta2")
        # phi2: even key-chunks on partitions 0:64, odd on 64:128;
        # free col block j holds key chunks (2j, 2j+1)
        phi2 = big.tile([P, N // 2], BF16, tag="phi2")
        gt = big.tile([P, MT, GT_STRIDE], F8, tag="gt")

        def theta_proj(q4):
            """theta for query cols q4*512.. (wqk = [W_theta^T | W_theta^T])."""
            ps = psQ.tile([P, QB], F32, tag="oproj")
            xc = xcols(q4 * QB, (q4 + 1) * QB)
            for k in range(2):
                nc.tensor.matmul(
                    ps,
                    lhsT=wk3_sb[:, 2 * k : 2 * k + 2, 0:P],
                    rhs=xc[:, 2 * k : 2 * k + 2, :],
                    start=(k == 0),
                    stop=(k == 1),
                    perf_mode=DR,
                )
            nc.vector.tensor_copy(theta2[:, q4 * QB : (q4 + 1) * QB], ps)

        def phi_proj(t):
            """phi2 cols [t*512,(t+1)*512) = key chunks 8t..8t+7: even chunks
            to partitions 0:64, odd to 64:128, via zero-padded lhsT halves
            accumulating into one PSUM tile."""
            ps = psQ.tile([P, QB], F32, tag="oproj")
            xt3 = xf[t].rearrange("p cb (pr two w) -> p cb pr two w", two=2, w=P)
            for k in range(2):
                nc.tensor.matmul(
                    ps,
                    lhsT=wk3_sb[:, 2 * k : 2 * k + 2, P : 2 * P],
                    rhs=xt3[:, 2 * k : 2 * k + 2, :, 0, :],
                    start=(k == 0),
                    stop=False,
                    perf_mode=DR,
                )
            for k in range(2):
                nc.tensor.matmul(
                    ps,
                    lhsT=wk3_sb[:, 2 * k : 2 * k + 2, 2 * P : 3 * P],
                    rhs=xt3[:, 2 * k : 2 * k + 2, :, 1, :],
                    start=False,
                    stop=(k == 1),
                    perf_mode=DR,
                )
            # fold the 1/SW^2 score descale into phi2 so exp keeps scale=1.0
            # (a non-unit activation scale costs ~20% on the Scalar engine)
            nc.vector.tensor_scalar_mul(
                phi2[:, t * QB : (t + 1) * QB], ps, 1.0 / (SW * SW)
            )

        def gt_proj(mi):
            """gt[m, c] = SG * sum_cin x[cin, m] * wg[cin, c], stored fp8."""
            ps = psP.tile([P, VD], F32, tag="small")
            xc = xcols(mi * P, (mi + 1) * P)
            for k in range(2):
                nc.tensor.matmul(
                    ps,
                    lhsT=xc[:, 2 * k : 2 * k + 2, :],
                    rhs=wk3_sb[:, 2 * k : 2 * k + 2, 3 * P : 3 * P + VD],
                    start=(k == 0),
                    stop=(k == 1),
                    perf_mode=DR,
                )
            nc.vector.tensor_copy(gt[:, mi, 0:VD], ps)

        # ---- scores + exp: pairs of key-chunks -> one 1024-wide exp ----
        def scores_pair(b, et, j):
            """exp(scores^T)*2^-EXP_SHIFT (fp8) for query block b, key chunks
            2j, 2j+1 (one row-group-packed matmul pair, one exp)."""
            ps = psS.tile([P, 2 * QB], F32, tag="scores", name=f"sc{b}_{j}")
            nc.tensor.matmul(
                ps[:, 0:QB],
                lhsT=phi2[0:KD, j * P : (j + 1) * P],
                rhs=theta2[0:KD, b * QB : (b + 1) * QB],
                start=True,
                stop=True,
                tile_position=(0, 0),
            )
            nc.tensor.matmul(
                ps[:, QB : 2 * QB],
                lhsT=phi2[KD:P, j * P : (j + 1) * P],
                rhs=theta2[KD:P, b * QB : (b + 1) * QB],
                start=True,
                stop=True,
                tile_position=(KD, 0),
            )
            nc.scalar.activation(
                out=et[:, 2 * j : 2 * j + 2, :],
                in_=ps.rearrange("p (k w) -> p k w", k=2),
                func=mybir.ActivationFunctionType.Exp,
                bias=exp_bias,
            )

        def new_et(b):
            return eb.tile([P, MT, QB], F8, tag="expT", name=f"et{b}")

        # emit per x-column-tile so compute unblocks as each DMA lands;
        # phi/scores lead each tile (they feed the exp stream), gt_proj
        # trails one tile behind to fill PE gaps without delaying scores
        et0 = new_et(0)
        for t in range(NCOL):
            if t == 0:
                # block-0 scores only read theta2 cols 0:512, so theta(1)
                # can wait until after the first score pairs are in flight
                theta_proj(0)
            elif t == 1:
                theta_proj(2)
                theta_proj(3)
            phi_proj(t)
            for j in range(4 * t, 4 * t + 4):
                scores_pair(0, et0, j)
            if t == 0:
                theta_proj(1)
            if t == 0:
                # gt denominator column; also needed before any PV
                nc.vector.memset(gt[:, :, VD : VD + 1], ONES)
            else:
                for mi in range(8 * (t - 1), 8 * t):
                    gt_proj(mi)
        for mi in range(8 * (NCOL - 1), 8 * NCOL):
            gt_proj(mi)

        def pv_block(b, et, et_next):
            omidT = wk.tile([P, NQB, VD], BF16, tag="omidT")
            omid = wk.tile([P, 2, QB], F8, tag="omid")

            def transpose_qc(qc):
                # [queries, 256] -> [256, queries]
                for oc2 in range(2):
                    pst = psQ.tile([P, P], BF16, tag="oproj")
                    nc.tensor.transpose(
                        pst, omidT[:, qc, oc2 * P : (oc2 + 1) * P], ident
                    )
                    nc.vector.tensor_copy(omid[:, oc2, qc * P : (qc + 1) * P], pst)

            for qc in range(NQB):
                # next block's score pairs, interleaved 1:4 with the PV
                # matmuls so the scalar engine's exp stream never starves
                pso = psP.tile([P, VD + 1], F32, tag="small")
                for j2 in range(MT // 2):
                    if j2 % 4 == 0 and et_next is not None:
                        scores_pair(b + 1, et_next, 4 * qc + j2 // 4)
                    nc.tensor.matmul(
                        pso,
                        lhsT=et[:, 2 * j2 : 2 * j2 + 2, qc * P : (qc + 1) * P],
                        rhs=gt[:, 2 * j2 : 2 * j2 + 2, 0 : VD + 1],
                        start=(j2 == 0),
                        stop=(j2 == MT // 2 - 1),
                        perf_mode=DR,
                    )
                rec = recp.tile([P, 1], F32, tag="rec")
                nc.vector.reciprocal(rec, pso[:, VD : VD + 1])
                nc.vector.tensor_scalar_mul(omidT[:, qc, :], pso[:, 0:VD], rec)
                if qc > 0:
                    transpose_qc(qc - 1)  # deps long met -> no PE stall
            transpose_qc(NQB - 1)
            # output projection + residual
            for oc in range(CB):
                psq = psQ.tile([P, QB], F32, tag="oproj")
                nc.tensor.matmul(
                    psq,
                    lhsT=wo_sb[:, 0:2, oc * P : (oc + 1) * P],
                    rhs=omid[:, 0:2, :],
                    start=True,
                    stop=True,
                    perf_mode=DR,
                )
                ot = outp.tile([P, QB], BF16, tag="out")
                nc.vector.scalar_tensor_tensor(
                    out=ot,
                    in0=psq,
                    scalar=resid_scale,
                    in1=xq[:, oc, b * QB : (b + 1) * QB],
                    op0=mybir.AluOpType.mult,
                    op1=mybir.AluOpType.add,
                )
                nc.sync.dma_start(out=out_r[:, oc, b * QB : (b + 1) * QB], in_=ot)

        et = et0
        for b in range(NQB):
            et_next = new_et(b + 1) if b + 1 < NQB else None
            pv_block(b, et, et_next)
            et = et_next

    nc.compile()
    return nc


_CACHE: dict = {}


def _get_nc(gamma: float) -> bass.Bass:
    if gamma not in _CACHE:
        _CACHE[gamma] = build_nc(gamma)
    return _CACHE[gamma]


def _prep_in_maps(x, W_theta, W_phi, W_g, W_o):
    x = np.ascontiguousarray(np.asarray(x, dtype=np.float32))
    bf16 = ml_dtypes.bfloat16
    f8 = ml_dtypes.float8_e4m3fn
    wth = np.asarray(W_theta, np.float32).T * SW
    wphT = np.asarray(W_phi, np.float32).T * SW
    wk3 = np.zeros((C, 5 * P), np.float32)
    wk3[:, 0:KD] = wth
    wk3[:, KD : 2 * KD] = wth
    wk3[:, P : P + KD] = wphT
    wk3[:, 2 * P + KD : 3 * P] = wphT
    wk3[:, 3 * P : 3 * P + VD] = np.asarray(W_g, np.float32).T * SG
    wk3 = np.ascontiguousarray(wk3).astype(f8)
    wo = np.ascontiguousarray(np.asarray(W_o, np.float32).T * SO).astype(f8)
    in_maps = []
    for core in range(8):
        b, h = divmod(core, 2)
        xb = x[b]
        x_perm = np.ascontiguousarray(
            np.concatenate(
                [xb[:, h * NQ : (h + 1) * NQ], xb[:, (1 - h) * NQ : (2 - h) * NQ]],
                axis=1,
            )
        )
        in_maps.append(
            {
                "x": x_perm.astype(f8),
                "xq": x_perm[:, 0:NQ].astype(bf16),
                "wk3": wk3,
                "wo": wo,
            }
        )
    return in_maps


def _run(x, W_theta, W_phi, W_g, W_o, gamma, trace=False):
    nc = _get_nc(float(gamma))
    in_maps = _prep_in_maps(x, W_theta, W_phi, W_g, W_o)
    # the first execution of a fresh NEFF occasionally hits a transient
    # NRT_EXEC_UNIT_UNRECOVERABLE on this fabric; a retry recovers it
    last_err = None
    for attempt in range(3):
        try:
            res = run_bass_kernel_spmd(nc, in_maps, list(range(8)), trace=trace)
            break
        except Exception as e:  # noqa: BLE001 - device-side flake, retry
            last_err = e
            import time

            time.sleep(2.0)
    else:
        raise last_err
    out = np.empty((4, C, N), np.float32)
    for core in range(8):
        b, h = divmod(core, 2)
        out[b][:, h * NQ : (h + 1) * NQ] = res.results[core]["out"].astype(np.float32)
    return out, res


def kernel(x, W_theta, W_phi, W_g, W_o, gamma):
    out, _ = _run(x, W_theta, W_phi, W_g, W_o, gamma)
    return out


# revision 52
# speedup vs baseline: 1.0061x; 1.0061x over previous
"""Trainium2 Bass kernel for the non-local attention block (nn_Attention_79809082295188).

Reference computation (per batch b of 4, C=512 channels, N=4096 positions):
    theta = W_theta @ x          [64, N]
    phi   = W_phi @ x            [64, N]
    g     = W_g @ x              [256, N]
    scores[n, m] = theta[:, n] . phi[:, m]
    beta = softmax(scores, axis=m)
    o_mid[c, n] = sum_m g[c, m] beta[n, m]
    out = gamma * (W_o @ o_mid) + x

Sharding: 8 shards = batch(4) x query-half(2). Each core receives its batch's
full x with its own query half permuted to the FIRST 2048 columns (key order is
irrelevant to softmax attention), computes attention for those 2048 queries
against all 4096 keys, and writes a [512, 2048] output chunk.

On-core dataflow (fp8 DoubleRow everywhere the PE allows it):
  - x ships as fp8e4m3 (2MB) for every projection; a bf16 copy of the core's
    own query half (xq) ships only for the residual add. Output is bf16.
  - theta/phi/g projections run fp8 DoubleRow: channel-block pairs ride the
    interleave dim, halving the moving-element cost vs bf16. Weights are
    pre-scaled by 64 on the host so fp8e4m3 stays in its normal range; the
    theta/phi scale is undone by a 1/4096 fold into the phi2 PSUM->SBUF copy
    (NOT via the exp activation's scale param -- a non-unit activation scale
    measured ~20% slower per exp on HW), the g/o scales cancel in the final
    residual multiplier gamma/2048.
  - wqk/wph/wg ship as ONE packed dram param (wk3): each dma_start costs
    ~0.7us of serialized Sync-engine descriptor generation, and the input
    DMA chain gates the pipeline head. Warmup matmuls (16 x 256-col) bridge
    the PE from boot to first-dependency-arrival; any PE idle gap in the
    head drops the clock (observed 630-690ns vs 380ns for the same matmul).
  - scores are computed TRANSPOSED ([keys m on partitions, queries n free])
    in bf16 with two 64-deep key-chunk matmuls packed onto the 128-row PE via
    tile_position row groups (measured: the pair co-streams, ~213ns/512q).
  - softmax denominator comes for free: a constant column appended to g^T
    makes column 256 of the PV matmul output equal 2*sum_m exp(scores_T[m,n]),
    which also lands o_mid at 32x (safe mid-range for its fp8 recast).
  - max-subtraction is skipped: scores are in [-11, 11], exp() is safe in fp32;
    exp is emitted as exp(s)*2^-9 so it fits fp8e4 range.
  - normalization is a per-partition scalar multiply, then a PE transpose of
    the [queries, 256] result back to [channels, queries] for the fp8-DR
    output projection. (A transpose-free oproj via is_transpose with a
    non-identity rhs does NOT work: the transpose path moves data without
    MACs, so it cannot contract -- produces garbage.)
"""

import sys

sys.path.insert(0, "/opt/trn_rl_repo")

from contextlib import ExitStack

import numpy as np
import ml_dtypes

import concourse.bass as bass
import concourse.bacc as bacc
import concourse.tile as tile
from concourse import mybir
from concourse.bass_utils import run_bass_kernel_spmd
from concourse.masks import make_identity

F32 = mybir.dt.float32
BF16 = mybir.dt.bfloat16
F8 = mybir.dt.float8e4
DR = mybir.MatmulPerfMode.DoubleRow

# exp() is emitted as exp(s)*2^-EXP_SHIFT so it fits fp8e4 range (max ~240
# vs exp(score_max~11) ~ 60000); the scale cancels in the softmax ratio.
EXP_SHIFT = 9
EXP_BIAS = -float(EXP_SHIFT) * 0.6931471805599453
GT_STRIDE = 272  # g^T row stride in fp8 bytes: 257 columns padded to %16==0

SW = 64.0        # host scale on W_theta/W_phi (fp8 normal range)
SG = 64.0        # host scale on W_g
SO = 64.0        # host scale on W_o
ONES = 2.0       # gt denominator column value; omid lands at SG/ONES = 32x

C = 512          # channels
N = 4096         # sequence positions (keys per core)
P = 128          # partitions
CB = C // P      # 4 channel blocks
KD = 64          # theta/phi dim (C/8)
VD = 256         # g dim (C/2)
NQ = 2048        # queries per core
QB = 512         # query block
NQB = NQ // QB   # 4 query blocks
MT = N // P      # 32 key tiles
NCOL = 4         # x column tiles (for DMA/compute overlap)
COLW = N // NCOL # 1024
N_WARMUP = 16    # PE warmup matmuls to ride out the input DMA + HAM cold clock


def build_nc(gamma: float) -> bass.Bass:
    resid_scale = float(gamma) / (SO * SG / ONES)
    nc = bacc.Bacc(
        "TRN2",
        target_bir_lowering=False,
        debug=False,
        enable_asserts=False,
        num_devices=8,
    )
    x_in = nc.declare_dram_parameter("x", [C, N], F8, isOutput=False)
    xq_in = nc.declare_dram_parameter("xq", [C, NQ], BF16, isOutput=False)
    # wk3 packs [wqk | wph | wg] along columns so ONE dma_start covers all
    # early weights (each dma_start costs ~0.7us of serialized Sync-engine
    # descriptor generation, which was gating the pipeline head).
    #   cols   0:128  wqk: [W_theta^T | W_theta^T]
    #   cols 128:384  wph: [W_phi^T | 0] then [0 | W_phi^T] -- lets the
    #     even/odd key-chunk projections land on partitions 0:64 / 64:128 of
    #     one PSUM tile via accumulation (walrus rejects col-tiled dst base 64)
    #   cols 384:640  wg
    wk3_in = nc.declare_dram_parameter("wk3", [C, 5 * P], F8, isOutput=False)
    wo_in = nc.declare_dram_parameter("wo", [VD, C], F8, isOutput=False)
    out_ext = nc.declare_dram_parameter("out", [C, NQ], BF16, isOutput=True)

    x_r = x_in.rearrange("(cb p) (j w) -> p cb j w", p=P, w=COLW)
    xq_r = xq_in.rearrange("(cb p) n -> p cb n", p=P)
    out_r = out_ext.rearrange("(cb p) n -> p cb n", p=P)

    with tile.TileContext(nc) as tc, ExitStack() as ctx:
        const = ctx.enter_context(tc.tile_pool(name="const", bufs=1))
        big = ctx.enter_context(tc.tile_pool(name="big", bufs=1))
        eb = ctx.enter_context(tc.tile_pool(name="eb", bufs=2))
        wk = ctx.enter_context(tc.tile_pool(name="wk", bufs=2))
        recp = ctx.enter_context(tc.tile_pool(name="recp", bufs=4))
        outp = ctx.enter_context(tc.tile_pool(name="outp", bufs=4))
        # PSUM budget (8 banks): scores pairs 2x2 + small 2 + oproj 2
        psS = ctx.enter_context(tc.tile_pool(name="psS", bufs=2, space="PSUM"))
        psP = ctx.enter_context(tc.tile_pool(name="psP", bufs=2, space="PSUM"))
        psQ = ctx.enter_context(tc.tile_pool(name="psQ", bufs=2, space="PSUM"))

        # ---- PE warmup: keep TensorE busy during input DMA so HAM unthrottles
        dummy = const.tile([P, QB], BF16, tag="dummy")
        nc.gpsimd.memset(dummy, 0.0)
        # load the exp table-set during the DMA window, not at first real exp
        warm_exp = const.tile([P, 1], F32, tag="warm_exp")
        nc.scalar.activation(
            out=warm_exp,
            in_=dummy[:, 0:1],
            func=mybir.ActivationFunctionType.Exp,
        )
        for i in range(N_WARMUP):
            psw = psS.tile([P, 2 * QB], F32, tag="scores")
            nc.tensor.matmul(
                psw[:, 0 : 2 * P],
                lhsT=dummy[:, 0:P],
                rhs=dummy[:, 0 : 2 * P],
                start=True,
                stop=True,
            )

        # ---- inputs: interleave x column tiles with the weights so the
        # first projection work unblocks as early as possible (wo last) ----
        xf = [
            big.tile([P, CB, COLW], F8, tag=f"xf{j}", name=f"xf{j}")
            for j in range(NCOL)
        ]
        xq = big.tile([P, CB, NQ], BF16, tag="xq")
        wk3_sb = const.tile([P, CB, 5 * P], F8, tag="wk3")
        wo_sb = const.tile([P, 2, C], F8, tag="wo")

        # wk3 (small) first so its transfer doesn't queue behind xf0's 1.5us
        nc.sync.dma_start(out=wk3_sb, in_=wk3_in.rearrange("(cb p) k -> p cb k", p=P))
        nc.sync.dma_start(out=xf[0], in_=x_r[:, :, 0, :])
        nc.sync.dma_start(out=xf[1], in_=x_r[:, :, 1, :])
        nc.sync.dma_start(out=xf[2], in_=x_r[:, :, 2, :])
        nc.sync.dma_start(out=xf[3], in_=x_r[:, :, 3, :])
        nc.sync.dma_start(out=wo_sb, in_=wo_in.rearrange("(cb p) k -> p cb k", p=P))
        nc.sync.dma_start(out=xq, in_=xq_r)
        ident = const.tile([P, P], BF16, tag="ident")
        make_identity(nc, ident)
        exp_bias = const.tile([P, 1], F32, tag="exp_bias")
        nc.vector.memset(exp_bias, EXP_BIAS)

        def xcols(lo, hi):
            """AP for x columns [lo, hi) -- must lie within one column tile."""
            j = lo // COLW
            assert hi <= (j + 1) * COLW
            return xf[j][:, :, lo - j * COLW : hi - j * COLW]

        # theta duplicated on both partition halves (for row-packed QK^T)
        theta2 = big.tile([P, NQ], BF16, tag="theta2")
        # phi2: even key-chunks on partitions 0:64, odd on 64:128;
        # free col block j holds key chunks (2j, 2j+1)
        phi2 = big.tile([P, N // 2], BF16, tag="phi2")
        gt = big.tile([P, MT, GT_STRIDE], F8, tag="gt")

        def theta_proj(q4):
            """theta for query cols q4*512.. (wqk = [W_theta^T | W_theta^T])."""
            ps = psQ.tile([P, QB], F32, tag="oproj")
            xc = xcols(q4 * QB, (q4 + 1) * QB)
            for k in range(2):
                nc.tensor.matmul(
                    ps,
                    lhsT=wk3_sb[:, 2 * k : 2 * k + 2, 0:P],
                    rhs=xc[:, 2 * k : 2 * k + 2, :],
                    start=(k == 0),
                    stop=(k == 1),
                    perf_mode=DR,
                )
            nc.vector.tensor_copy(theta2[:, q4 * QB : (q4 + 1) * QB], ps)

        def phi_proj(t):
            """phi2 cols [t*512,(t+1)*512) = key chunks 8t..8t+7: even chunks
            to partitions 0:64, odd to 64:128, via zero-padded lhsT halves
            accumulating into one PSUM tile."""
            ps = psQ.tile([P, QB], F32, tag="oproj")
            xt3 = xf[t].rearrange("p cb (pr two w) -> p cb pr two w", two=2, w=P)
            for k in range(2):
                nc.tensor.matmul(
                    ps,
                    lhsT=wk3_sb[:, 2 * k : 2 * k + 2, P : 2 * P],
                    rhs=xt3[:, 2 * k : 2 * k + 2, :, 0, :],
                    start=(k == 0),
                    stop=False,
                    perf_mode=DR,
                )
            for k in range(2):
                nc.tensor.matmul(
                    ps,
                    lhsT=wk3_sb[:, 2 * k : 2 * k + 2, 2 * P : 3 * P],
                    rhs=xt3[:, 2 * k : 2 * k + 2, :, 1, :],
                    start=False,
                    stop=(k == 1),
                    perf_mode=DR,
                )
            # fold the 1/SW^2 score descale into phi2 so exp keeps scale=1.0
            # (a non-unit activation scale costs ~20% on the Scalar engine)
            nc.vector.tensor_scalar_mul(
                phi2[:, t * QB : (t + 1) * QB], ps, 1.0 / (SW * SW)
            )

        def gt_proj(mi):
            """gt[m, c] = SG * sum_cin x[cin, m] * wg[cin, c], stored fp8."""
            ps = psP.tile([P, VD], F32, tag="small")
            xc = xcols(mi * P, (mi + 1) * P)
            for k in range(2):
                nc.tensor.matmul(
                    ps,
                    lhsT=xc[:, 2 * k : 2 * k + 2, :],
                    rhs=wk3_sb[:, 2 * k : 2 * k + 2, 3 * P : 3 * P + VD],
                    start=(k == 0),
                    stop=(k == 1),
                    perf_mode=DR,
                )
            nc.vector.tensor_copy(gt[:, mi, 0:VD], ps)

        # ---- scores + exp: pairs of key-chunks -> one 1024-wide exp ----
        def scores_pair(b, et, j):
            """exp(scores^T)*2^-EXP_SHIFT (fp8) for query block b, key chunks
            2j, 2j+1 (one row-group-packed matmul pair, one exp)."""
            ps = psS.tile([P, 2 * QB], F32, tag="scores", name=f"sc{b}_{j}")
            nc.tensor.matmul(
                ps[:, 0:QB],
                lhsT=phi2[0:KD, j * P : (j + 1) * P],
                rhs=theta2[0:KD, b * QB : (b + 1) * QB],
                start=True,
                stop=True,
                tile_position=(0, 0),
            )
            nc.tensor.matmul(
                ps[:, QB : 2 * QB],
                lhsT=phi2[KD:P, j * P : (j + 1) * P],
                rhs=theta2[KD:P, b * QB : (b + 1) * QB],
                start=True,
                stop=True,
                tile_position=(KD, 0),
            )
            nc.scalar.activation(
                out=et[:, 2 * j : 2 * j + 2, :],
                in_=ps.rearrange("p (k w) -> p k w", k=2),
                func=mybir.ActivationFunctionType.Exp,
                bias=exp_bias,
            )

        def new_et(b):
            return eb.tile([P, MT, QB], F8, tag="expT", name=f"et{b}")

        # emit per x-column-tile so compute unblocks as each DMA lands;
        # phi/scores lead each tile (they feed the exp stream), gt_proj
        # trails one tile behind to fill PE gaps without delaying scores
        et0 = new_et(0)
        for t in range(NCOL):
            if t < 2:
                theta_proj(2 * t)
                theta_proj(2 * t + 1)
            phi_proj(t)
            for j in range(4 * t, 4 * t + 4):
                scores_pair(0, et0, j)
            if t == 0:
                # gt denominator column; also needed before any PV
                nc.vector.memset(gt[:, :, VD : VD + 1], ONES)
            else:
                for mi in range(8 * (t - 1), 8 * t):
                    gt_proj(mi)
        for mi in range(8 * (NCOL - 1), 8 * NCOL):
            gt_proj(mi)

        def pv_block(b, et, et_next):
            omidT = wk.tile([P, NQB, VD], BF16, tag="omidT")
            omid = wk.tile([P, 2, QB], F8, tag="omid")

            def transpose_qc(qc):
                # [queries, 256] -> [256, queries]
                for oc2 in range(2):
                    pst = psQ.tile([P, P], BF16, tag="oproj")
                    nc.tensor.transpose(
                        pst, omidT[:, qc, oc2 * P : (oc2 + 1) * P], ident
                    )
                    nc.vector.tensor_copy(omid[:, oc2, qc * P : (qc + 1) * P], pst)

            for qc in range(NQB):
                # next block's score pairs, interleaved 1:4 with the PV
                # matmuls so the scalar engine's exp stream never starves
                pso = psP.tile([P, VD + 1], F32, tag="small")
                for j2 in range(MT // 2):
                    if j2 % 4 == 3 and et_next is not None:
                        scores_pair(b + 1, et_next, 4 * qc + j2 // 4)
                    nc.tensor.matmul(
                        pso,
                        lhsT=et[:, 2 * j2 : 2 * j2 + 2, qc * P : (qc + 1) * P],
                        rhs=gt[:, 2 * j2 : 2 * j2 + 2, 0 : VD + 1],
                        start=(j2 == 0),
                        stop=(j2 == MT // 2 - 1),
                        perf_mode=DR,
                    )
                rec = recp.tile([P, 1], F32, tag="rec")
                nc.vector.reciprocal(rec, pso[:, VD : VD + 1])
                nc.vector.tensor_scalar_mul(omidT[:, qc, :], pso[:, 0:VD], rec)
                if qc > 0:
                    transpose_qc(qc - 1)  # deps long met -> no PE stall
            transpose_qc(NQB - 1)
            # output projection + residual
            for oc in range(CB):
                psq = psQ.tile([P, QB], F32, tag="oproj")
                nc.tensor.matmul(
                    psq,
                    lhsT=wo_sb[:, 0:2, oc * P : (oc + 1) * P],
                    rhs=omid[:, 0:2, :],
                    start=True,
                    stop=True,
                    perf_mode=DR,
                )
                ot = outp.tile([P, QB], BF16, tag="out")
                nc.vector.scalar_tensor_tensor(
                    out=ot,
                    in0=psq,
                    scalar=resid_scale,
                    in1=xq[:, oc, b * QB : (b + 1) * QB],
                    op0=mybir.AluOpType.mult,
                    op1=mybir.AluOpType.add,
                )
                nc.sync.dma_start(out=out_r[:, oc, b * QB : (b + 1) * QB], in_=ot)

        et = et0
        for b in range(NQB):
            et_next = new_et(b + 1) if b + 1 < NQB else None
            pv_block(b, et, et_next)
            et = et_next

    nc.compile()
    return nc


_CACHE: dict = {}


def _get_nc(gamma: float) -> bass.Bass:
    if gamma not in _CACHE:
        _CACHE[gamma] = build_nc(gamma)
    return _CACHE[gamma]


def _prep_in_maps(x, W_theta, W_phi, W_g, W_o):
    x = np.ascontiguousarray(np.asarray(x, dtype=np.float32))
    bf16 = ml_dtypes.bfloat16
    f8 = ml_dtypes.float8_e4m3fn
    wth = np.asarray(W_theta, np.float32).T * SW
    wphT = np.asarray(W_phi, np.float32).T * SW
    wk3 = np.zeros((C, 5 * P), np.float32)
    wk3[:, 0:KD] = wth
    wk3[:, KD : 2 * KD] = wth
    wk3[:, P : P + KD] = wphT
    wk3[:, 2 * P + KD : 3 * P] = wphT
    wk3[:, 3 * P : 3 * P + VD] = np.asarray(W_g, np.float32).T * SG
    wk3 = np.ascontiguousarray(wk3).astype(f8)
    wo = np.ascontiguousarray(np.asarray(W_o, np.float32).T * SO).astype(f8)
    in_maps = []
    for core in range(8):
        b, h = divmod(core, 2)
        xb = x[b]
        x_perm = np.ascontiguousarray(
            np.concatenate(
                [xb[:, h * NQ : (h + 1) * NQ], xb[:, (1 - h) * NQ : (2 - h) * NQ]],
                axis=1,
            )
        )
        in_maps.append(
            {
                "x": x_perm.astype(f8),
                "xq": x_perm[:, 0:NQ].astype(bf16),
                "wk3": wk3,
                "wo": wo,
            }
        )
    return in_maps


def _run(x, W_theta, W_phi, W_g, W_o, gamma, trace=False):
    nc = _get_nc(float(gamma))
    in_maps = _prep_in_maps(x, W_theta, W_phi, W_g, W_o)
    # the first execution of a fresh NEFF occasionally hits a transient
    # NRT_EXEC_UNIT_UNRECOVERABLE on this fabric; a retry recovers it
    last_err = None
    for attempt in range(3):
        try:
            res = run_bass_kernel_spmd(nc, in_maps, list(range(8)), trace=trace)
            break
        except Exception as e:  # noqa: BLE001 - device-side flake, retry
            last_err = e
            import time

            time.sleep(2.0)
    else:
        raise last_err
    out = np.empty((4, C, N), np.float32)
    for core in range(8):
        b, h = divmod(core, 2)
        out[b][:, h * NQ : (h + 1) * NQ] = res.results[core]["out"].astype(np.float32)
    return out, res


def kernel(x, W_theta, W_phi, W_g, W_o, gamma):
    out, _ = _run(x, W_theta, W_phi, W_g, W_o, gamma)
    return out
